# revision 48
# baseline (speedup 1.0000x reference)
"""Trainium2 Bass kernel for windowless 3D relative-position attention.

Full-input contract: kernel(**inputs) takes the unsharded numpy inputs and
returns the full [4, 2048, 256] output. Internally shards across 8 NeuronCores
as (batch b = core//2) x (head-group g = core%2, 4 heads each).

v2 design, measured ~170us (ACT-exp is the pacemaker at ~1.03us per
FD=1024 instruction; the PE mostly runs cold at 1.2 GHz due to HAM, so
per-ACT-slot PE work is kept under ~1.3us):
  - expBT resident in SBUF ([128, 16, 2048] fp16) loaded once. DMA order
    matters: x cols 0:1024 + w_qk + wv + ebt0 BEFORE x second halves and
    the ebt bulk -- wv queued behind the 8.4MB ebt stream slides the whole
    v-projection into quarter 0 and starves ACT (DMA transfers of queued
    descriptors interleave round-robin, so a late-queued small transfer
    finishes late no matter its size).
  - scores per head-pair tile [128, 2, 512] fp32 (2 PSUM banks), one shared
    tag ring of 3 bufs (6 banks) -> the 4 per-m score matmuls (row bands
    0..3 via tile_position) fly concurrently while ACT drains earlier
    tiles.
  - ONE ACT exp per head-pair (FD=1024, PSUM->SBUF fp16), writing halves of
    a [128, 4, 512] aw tile; ONE DVE mul (FD=2048, 2x mode) applies
    exp(bias) to all 4 heads per m-tile (eb broadcast via stride-0 AP).
    aw/aw2 rings are 6 deep so late ebt arrivals never stall the exps.
  - AV lhsT is [128 keys, 64]: cols 0:32 = v, 32:64 = ones -> the softmax
    denominator comes out REPLICATED on 32 psum partitions; normalization is
    reciprocal_approx_fast per oa bank + one fp32 mul per head (num stays
    in PSUM: the walrus verifier requires all SBUF inputs of a
    tensor_tensor to share a start partition; PSUM inputs are exempt).
  - oa: 2 heads per bank at PE column offsets 0/64 (2-way AV concurrency),
    2 banks per quarter, single-buffered. PSUM total = 6 (sc ring) + 2 = 8.
  - AV matmuls trail the score stream by one m-slot and are carried ACROSS
    quarter boundaries (issued at the next quarter's slot 0, after the
    hoisted next-quarter scores but before the normalize reads the old oa
    banks), so neither the oa WAR nor the trailing AVs ever block score
    issue in the in-order PE queue (ACT never starves).
  - quarter-q normalize + out-projection run as 7 single-slot tasks popped
    during quarter q+1: recip, 2 nmuls, 2x2 out-proj matmuls into a cycled
    sc-ring tile (2 banks hold all four [128, 256] n-tile outputs), 2x2
    PSUM->SBUF copies, then one batched [512, 256] output DMA on the sync
    queue -- outputs stream during attention instead of piling up at the
    tail.
  - the last m-tile of quarter 3 splits exp/mul/AV per head-pair so the
    final normalize starts ~1us earlier; tail out-proj copies alternate
    scalar/vector engines.
  - dummy exp activation issued at t~0 so the ~2.7us ACT table load hides
    under the input DMAs; prologue interleaves qk-proj chunks, v-proj
    tiles and the first five m-tiles' scores/exps so ACT starts ~15us in
    (~7us of that is fixed framework preamble).

The bias add is factored through the exponential: exp(s+bias) =
exp(s)*exp(bias), with exp(bias.T - C_SHIFT) precomputed on host in fp16
(C_SHIFT keeps products in fp16 range; it cancels in the softmax ratio).

Hardware constraints learned (do not re-attempt blindly): matmul PSUM out
is limited to one 2KB bank (512 fp32 cols); engine APs must start on
partition 0/32/64/96 and fit the quadrant; DVE cannot shift partitions
(two SBUF inputs must share a start partition); DMA cannot touch PSUM;
DMA partition stride must be 1 (stride-0 broadcast only from DRAM); tile
pools release LIFO; tile() with name= but no tag= makes the name the tag
(one ring per name!); the Tile scheduler reorders by sim-readiness +
priority, so the in-order engine queues stall when the sim mispredicts a
DMA arrival -- structure deps so nothing DMA-gated sits ahead of ready
work; walrus ldw-opt is incompatible with Bass IR; fp8 DoubleRow gives no
column-rate gain at K<128; GPSIMD DGE adds latency in dependency-critical
chains and GPSIMD cannot touch PSUM; HAM keeps PE at 1.2 GHz unless
continuously busy >=3.4us -- the ACT-gated cadence's micro-idles make it
oscillate, so budget PE work at the 1.2 GHz rate. Scheduling here is
HYPERSENSITIVE: compacting boundary tasks, deepening rings past 6, or
finer DMA splits all measured WORSE (171-181us vs 170); measure every
change.
"""

import os
import sys
from contextlib import ExitStack

import numpy as np

sys.path.insert(0, "/opt/trn_rl_repo")

import concourse.bass as bass
import concourse.bacc as bacc
import concourse.tile as tile
from concourse import mybir
from concourse.bass_utils import run_bass_kernel_spmd

# Problem constants (hardcoded per contract)
B = 4
N = 2048
INP = 256
OUP = 256
HEADS = 8
DIM_HEAD = 32
SCALE = DIM_HEAD ** -0.5
HL = 4            # heads per core
MT = N // 128     # 16 m-tiles (keys)
NQ = 4            # 512-wide n (query) quarters
NQW = 512
C_SHIFT = 4.0

f32 = mybir.dt.float32
f16 = mybir.dt.float16

_LAST = {"exec_time_ns": None}


def _build_nc():
    nc = bacc.Bacc("TRN2", target_bir_lowering=False, debug=False)
    xT_d = nc.dram_tensor("xT", [2, 128, N], f16, kind="ExternalInput")
    wqk_d = nc.dram_tensor("w_qk", [2, 128, 256], f16, kind="ExternalInput")
    wv_d = nc.dram_tensor("w_v", [2, 128, 128], f16, kind="ExternalInput")
    wout_d = nc.dram_tensor("w_out2", [128, 256], f16, kind="ExternalInput")
    ebt_d = nc.dram_tensor("expbt", [N, N], f16, kind="ExternalInput")
    out_d = nc.dram_tensor("partial", [N, OUP], f32, kind="ExternalOutput")

    with ExitStack() as ctx:
        tc = ctx.enter_context(tile.TileContext(nc))
        consts = ctx.enter_context(tc.tile_pool(name="consts", bufs=1))

        ebt = consts.tile([128, MT, N], f16)          # [m%128, mtile, n]
        xT = consts.tile([128, 2, N], f16)
        wqk = consts.tile([128, 2, 256], f16)
        wv = consts.tile([128, 2, 128], f16)
        woutd = consts.tile([128, 256], f16)
        qkT = consts.tile([128, 2, N], f16)           # [:,0,:]=qT  [:,1,:]=kT
        vsb = consts.tile([128, MT, HL, 64], f16)     # [key, mtile, head, v|ones]
        aoutT = consts.tile([128, N], f16)            # [(h,d), n] normalized
        dummy = consts.tile([128, 8], f32)

        # dummy exp right away: the ~2.7us ACT table load hides under DMAs
        nc.vector.memset(dummy[:], 1.0)
        nc.scalar.activation(
            out=dummy[:, 0:4], in_=dummy[:, 4:8],
            func=mybir.ActivationFunctionType.Exp, scale=1.0,
        )

        # DMA order: x cols 0:1024 (covers qk-proj ch0+ch1 and v-tiles 0-7)
        # + w_qk + wv first -- wv must NOT queue behind the ebt stream or
        # the whole v-projection slides into quarter 0 and starves ACT.
        # ebt0 lands before the first DVE bias-mul needs it; the rest
        # streams behind (the deep aw ring tolerates late muls).
        for kk in range(2):
            nc.sync.dma_start(out=xT[:, kk, 0:N // 2], in_=xT_d[kk, :, 0:N // 2])
        for kk in range(2):
            nc.sync.dma_start(out=wqk[:, kk, :], in_=wqk_d[kk])
        for kk in range(2):
            nc.sync.dma_start(out=wv[:, kk, :], in_=wv_d[kk])
        nc.sync.dma_start(out=ebt[:, 0, :], in_=ebt_d[0:128, :])
        for kk in range(2):
            nc.sync.dma_start(out=xT[:, kk, N // 2:N], in_=xT_d[kk, :, N // 2:N])
        nc.sync.dma_start(out=ebt[:, 1, :], in_=ebt_d[128:256, :])
        nc.sync.dma_start(out=woutd[:], in_=wout_d[:])
        for m in range(2, MT):
            nc.sync.dma_start(out=ebt[:, m, :], in_=ebt_d[m * 128:(m + 1) * 128, :])
        nc.gpsimd.memset(vsb[:], 1.0)

        # --- attention (projections interleaved into the early q0 stream) ---
        with tc.tile_pool(name="awp", bufs=6) as awp, \
             tc.tile_pool(name="aw2p", bufs=6) as aw2p, \
             tc.tile_pool(name="otp", bufs=2) as otp, \
             tc.tile_pool(name="recp", bufs=2) as recp:
          with tc.tile_pool(name="sps", bufs=3, space="PSUM") as sps:

            def issue_scores(m, ncol0):
                scs = []
                for hp in range(2):
                    sc = sps.tile([128, 2, NQW], f32, tag="sc",
                                  name=f"sc{hp}")
                    for hi in range(2):
                        hl = hp * 2 + hi
                        nc.tensor.matmul(
                            sc[:, hi, :],
                            lhsT=qkT[32 * hl:32 * (hl + 1), 1,
                                     m * 128:(m + 1) * 128],
                            rhs=qkT[32 * hl:32 * (hl + 1), 0,
                                    ncol0:ncol0 + NQW],
                            start=True, stop=True,
                            tile_position=(32 * hl, 0),
                        )
                    scs.append(sc)
                return scs

            def issue_act_mul(m, ncol0, scs):
                aw = awp.tile([128, HL, NQW], f16, tag="aw")
                for hp in range(2):
                    nc.scalar.activation(
                        out=aw[:, 2 * hp:2 * hp + 2, :], in_=scs[hp][:],
                        func=mybir.ActivationFunctionType.Exp,
                        scale=SCALE,
                    )
                ebs = ebt[:, m, ncol0:ncol0 + NQW]
                eb_b = bass.AP(
                    tensor=ebs.tensor, offset=ebs.offset,
                    ap=[ebs.ap[0], [0, HL], ebs.ap[1]],
                )
                aw2 = aw2p.tile([128, HL, NQW], f16, tag="aw2")
                nc.vector.tensor_mul(aw2[:], aw[:], eb_b)
                return aw2

            def issue_av(m, aw2, oa):
                for hp in range(2):
                    for hi in range(2):
                        hl = hp * 2 + hi
                        po = 64 * (hl % 2)
                        nc.tensor.matmul(
                            oa[hl // 2][po:po + 64, :],
                            lhsT=vsb[:, m, hl, :],
                            rhs=aw2[:, hl, :],
                            start=(m == 0), stop=(m == MT - 1),
                        )

            # prologue: qk-projection chunks, v-projection tiles and the
            # first six m-tiles' scores+exp of quarter 0 are interleaved so
            # ACT starts ~5us in and never gaps while the PE does the
            # projections. AVs for these tiles are deferred into the main
            # loop (av_back) and drained 2/slot.
            pro_scs = []      # scores tiles t0..t5
            pro_aw2 = []      # aw2 tiles t0..t4
            with tc.tile_pool(name="ppsq", bufs=2, space="PSUM") as ppsq:
                def qk_chunk(ch):
                    for mb in range(2):   # 0 -> q block, 1 -> k block
                        ps = ppsq.tile([128, 512], f32, tag="qkps",
                                       name=f"qkps{ch}_{mb}")
                        for kk in range(2):
                            nc.tensor.matmul(
                                ps[:],
                                lhsT=wqk[:, kk, mb * 128:(mb + 1) * 128],
                                rhs=xT[:, kk, ch * 512:(ch + 1) * 512],
                                start=(kk == 0), stop=(kk == 1),
                            )
                        nc.vector.tensor_copy(
                            out=qkT[:, mb, ch * 512:(ch + 1) * 512], in_=ps[:]
                        )

                def pro_tile(t):
                    if t >= 1:   # t-1's exp/mul before t's scores (sc ring
                        pro_aw2.append(issue_act_mul(t - 1, 0, pro_scs[t - 1]))
                    pro_scs.append(issue_scores(t, 0))

                qk_chunk(0)
                pro_tile(0)
                pro_tile(1)
                qk_chunk(1)
                pro_tile(2)
                qk_chunk(2)
                qk_chunk(3)
                pro_tile(3)

            with tc.tile_pool(name="ppsv", bufs=2, space="PSUM") as ppsv:
                def v_tile(nt):
                    ps = ppsv.tile([128, 128], f32, tag="vps",
                                   name=f"vps{nt}")
                    for kk in range(2):
                        nc.tensor.matmul(
                            ps[:],
                            lhsT=xT[:, kk, nt * 128:(nt + 1) * 128],
                            rhs=wv[:, kk, :],
                            start=(kk == 0), stop=(kk == 1),
                        )
                    nc.vector.tensor_copy(out=vsb[:, nt, :, 0:32], in_=ps[:])

                for nt in range(4):
                    v_tile(nt)
                pro_tile(4)
                for nt in range(4, MT):
                    v_tile(nt)

            with tc.tile_pool(name="oap", bufs=1, space="PSUM") as oap:

                def make_boundary_tasks(qq, oa_q, tail=False):
                    """Normalize + out-project quarter qq as 7 single-slot
                    tasks, interleaved into quarter qq+1's m-stream.

                    oa is read by recip (task 0) and the nmuls (tasks 1-2)
                    so quarter qq+1's AVs wait ~3 slots (drained 2/slot);
                    the out-projection goes into a freshly cycled sc-ring
                    tile (2 banks) so no extra PSUM is needed and PE-queue
                    stalls are bounded by the ring lookahead.

                    NOTE: nmul keeps the num operand in PSUM — the walrus
                    verifier requires all SBUF *inputs* of a tensor_tensor
                    to share a start partition (PSUM inputs are exempt)."""
                    recs = [recp.tile([128, NQW], f32, tag=f"rec{i}",
                                      name=f"rec{i}_{qq}") for i in range(2)]
                    ot = otp.tile([128, 4, OUP], f32, tag="ot",
                                  name=f"ot{qq}")
                    prj = [None]

                    def recip():
                        for i in range(2):
                            nc.vector.reciprocal_approx_fast(
                                out=recs[i][:], in_=oa_q[i][:]
                            )

                    def make_nmul(hp):
                        def nmul():
                            for hl in (2 * hp, 2 * hp + 1):
                                po = 64 * (hl % 2)
                                nc.vector.tensor_mul(
                                    aoutT[32 * hl:32 * hl + 32,
                                          qq * NQW:(qq + 1) * NQW],
                                    oa_q[hl // 2][po:po + 32, :],
                                    recs[hl // 2][po + 32:po + 64, :],
                                )
                        return nmul

                    def make_prjmm(jp):
                        def prjmm():
                            if prj[0] is None:
                                prj[0] = sps.tile([128, 2, NQW], f32,
                                                  tag="sc",
                                                  name=f"prj{qq}")
                            for j in (2 * jp, 2 * jp + 1):
                                nt = 4 * qq + j
                                pp = prj[0][:, j // 2,
                                            (j % 2) * OUP:(j % 2 + 1) * OUP]
                                nc.tensor.matmul(
                                    pp,
                                    lhsT=aoutT[:, nt * 128:(nt + 1) * 128],
                                    rhs=woutd[:],
                                    start=True, stop=True,
                                )
                        return prjmm

                    def make_pcopy(jp):
                        def pcopy():
                            for j in (2 * jp, 2 * jp + 1):
                                src = prj[0][:, j // 2,
                                             (j % 2) * OUP:(j % 2 + 1) * OUP]
                                if tail and j % 2 == 0:
                                    nc.scalar.copy(out=ot[:, j, :], in_=src)
                                else:
                                    nc.vector.tensor_copy(out=ot[:, j, :],
                                                          in_=src)
                            if jp == 1:
                                od = out_d[qq * 512:(qq + 1) * 512, :]
                                od4 = bass.AP(
                                    tensor=od.tensor, offset=od.offset,
                                    ap=[[OUP, 128], [OUP * 128, 4],
                                        [1, OUP]],
                                )
                                nc.sync.dma_start(out=od4, in_=ot[:])
                        return pcopy

                    return [recip, make_nmul(0), make_nmul(1),
                            make_prjmm(0), make_prjmm(1),
                            make_pcopy(0), make_pcopy(1)]

                tasks = []
                next_scs = None
                for q in range(NQ):
                    ncol0 = q * NQW
                    oa = [oap.tile([128, NQW], f32, tag=f"oa{i}",
                                   name=f"oa{i}_{q}")
                          for i in range(2)]

                    if q == 0:
                        # prologue pre-issued scores t0..t4, exp/mul t0..t3
                        scs = pro_scs[4]
                        av_back = [(t, pro_aw2[t], oa) for t in range(4)]
                        m0 = 4
                    else:
                        scs = next_scs
                        m0 = 0
                    for m in range(m0, MT):
                        if q == NQ - 1 and m == MT - 1:
                            # tail: split exp/mul/AV per head-pair so the
                            # final normalize can start ~2us earlier
                            for it in av_back:
                                issue_av(*it)
                            av_back = []
                            aw = awp.tile([128, HL, NQW], f16, tag="aw",
                                          name="aw_tail")
                            aw2 = aw2p.tile([128, HL, NQW], f16, tag="aw2",
                                            name="aw2_tail")
                            ebs = ebt[:, m, ncol0:ncol0 + NQW]
                            eb2 = bass.AP(
                                tensor=ebs.tensor, offset=ebs.offset,
                                ap=[ebs.ap[0], [0, 2], ebs.ap[1]],
                            )
                            for hp in range(2):
                                sl = slice(2 * hp, 2 * hp + 2)
                                nc.scalar.activation(
                                    out=aw[:, sl, :], in_=scs[hp][:],
                                    func=mybir.ActivationFunctionType.Exp,
                                    scale=SCALE,
                                )
                                nc.vector.tensor_mul(
                                    aw2[:, sl, :], aw[:, sl, :], eb2)
                                for hi in range(2):
                                    hl = 2 * hp + hi
                                    po = 64 * (hl % 2)
                                    nc.tensor.matmul(
                                        oa[hp][po:po + 64, :],
                                        lhsT=vsb[:, m, hl, :],
                                        rhs=aw2[:, hl, :],
                                        start=False, stop=True,
                                    )
                            continue
                        aw2 = issue_act_mul(m, ncol0, scs)
                        if m + 1 < MT:
                            scs = issue_scores(m + 1, ncol0)
                        elif q + 1 < NQ:
                            # next quarter's first scores go ahead of the
                            # carried AV backlog so ACT never gaps
                            next_scs = issue_scores(0, ncol0 + NQW)
                        if q > 0 and m == 0:
                            # drain the previous quarter's carried AVs now:
                            # they must be issued before task 0 (recip)
                            # reads the old oa banks
                            while av_back:
                                issue_av(*av_back.pop(0))
                        if tasks:
                            tasks.pop(0)()
                        av_back.append((m, aw2, oa))
                        if len(av_back) > 1:
                            issue_av(*av_back.pop(0))
                        if len(av_back) > 2 and m % 3 == 0:
                            issue_av(*av_back.pop(0))
                    # no flush: trailing AVs carry into the next quarter's
                    # slots (their oa tiles ride along in av_back)
                    tasks = make_boundary_tasks(q, oa, tail=(q == NQ - 1))

                # q3 boundary flush (inside the pool scopes)
                for t in tasks:
                    t()
                tasks = []
    nc.compile()
    return nc


_NC_CACHE = {}


def kernel(x, w_qkv, bias_table, w_out, b_out, relative_pos):
    x = np.asarray(x, np.float32)
    w_qkv = np.asarray(w_qkv, np.float32)
    bias_table = np.asarray(bias_table, np.float32)
    w_out = np.asarray(w_out, np.float32)
    b_out = np.asarray(b_out, np.float32)
    relative_pos = np.asarray(relative_pos, np.int32)

    bias = bias_table[relative_pos, 0]                       # [n, m]
    expBT = np.exp(bias.T - C_SHIFT).astype(np.float16)      # [m, n]
    expBT = np.ascontiguousarray(expBT)

    if "nc" not in _NC_CACHE:
        _NC_CACHE["nc"] = _build_nc()
    nc = _NC_CACHE["nc"]

    in_maps = []
    for c in range(8):
        b, g = c // 2, c % 2
        w_qk = np.concatenate(
            [w_qkv[:, g * 128:(g + 1) * 128],
             w_qkv[:, 256 + g * 128:256 + (g + 1) * 128]], axis=1)
        in_maps.append({
            "xT": np.ascontiguousarray(x[b].T).reshape(2, 128, N).astype(np.float16),
            "w_qk": np.ascontiguousarray(w_qk).reshape(2, 128, 256).astype(np.float16),
            "w_v": np.ascontiguousarray(
                w_qkv[:, 512 + g * 128:512 + (g + 1) * 128]
            ).reshape(2, 128, 128).astype(np.float16),
            "w_out2": np.ascontiguousarray(
                w_out[g * 128:(g + 1) * 128, :]
            ).astype(np.float16),
            "expbt": expBT,
        })

    trace = bool(os.environ.get("KERNEL_TRACE"))
    res = run_bass_kernel_spmd(nc, in_maps, list(range(8)), trace=trace)
    _LAST["exec_time_ns"] = res.exec_time_ns
    _LAST["results"] = res

    parts = [np.asarray(res.results[c]["partial"], np.float32) for c in range(8)]
    out = np.stack([parts[2 * b] + parts[2 * b + 1] + b_out for b in range(B)])
    return out.astype(np.float32)
